# revision 5
# baseline (speedup 1.0000x reference)
"""MoE (top-1 routing, capacity-dropped) forward on 8 Trainium2 NeuronCores.

Strategy (expert-parallel, per the sharding hint): the host computes the
top-1 gating (bit-identical to the reference's jax ops, on jax-CPU), which
*is* the sharding step -- tokens are dispatched along the expert axis E,
one expert per NeuronCore, with W1/b1/W2/b2 sharded along E and Wg applied
once on the full token set.  Each core runs its expert's FFN on C=1024
dispatched tokens in transposed [feature, token] layout:
    yT = W2[e].T @ relu(W1[e].T @ xT + b1[e]) + b2[e]
The host then scatters each expert's output rows back to token positions
scaled by the gate value (zeros for dropped tokens).

Kernel design notes (measured on HW via repeat-loop wall-clock slopes; the
PE roofline for this shape is 2*524288 cycles/core of fp32-class matmul):
  - bf16 datapath everywhere (PE rate identical to fp32r, but half the DMA
    bytes and datapath toggle power: the 8-core sustained stream is power-
    throttled, and bf16 measured ~9% faster than fp32r end-to-end)
  - bf16 quantization of x/W1/W2/h1 puts output rel err at ~3.7e-3
  - ~32 tiny warm-up matmuls run during the startup DMA wait so the PE HAM
    clock-gate reaches 8/8 before the real stream begins
  - x loads as two half-column waves across the three DMA queues so the
    first matmul group starts ~2.5us after launch
  - n-blocks of one output tile interleave in the k loop sharing one lhsT
    load (halves LDWEIGHTS traffic)
  - final output tile computes as 512/256/128/128-column blocks with the
    last stores on the two HWDGE queues so their HBM receipts overlap
  - unused capacity slots are padded with copies of real tokens, not zeros
    (the PE clock-gate watches datapath *activity*; all-zero data measured
    ~29% slower than random data on this silicon), and their outputs are
    discarded by the host combine

Hardcoded shapes: x [4, 2048, 1024], Wg [1024, 8], W1 [8, 1024, 4096],
b1 [8, 4096], W2 [8, 4096, 1024], b2 [8, 1024].
"""

import os
import numpy as np
from contextlib import ExitStack

try:
    from antenv.axon_hooks import get_axon_ntff_profile_hook  # noqa: F401
except Exception:
    os.environ.setdefault("BASS_NEVER_TRACE", "1")

from concourse import bacc, mybir, tile
from concourse.bass_utils import run_bass_kernel_spmd

B, S, H, F, E = 4, 2048, 1024, 4096, 8
T = B * S
C = -(-T // E)
P = 128
NB = 512
NCORES = 8
KC = 8

_DT = mybir.dt.bfloat16


def _build_expert_ffn(repeat: int = 1, warmup: int = 32):
    nc = bacc.Bacc("TRN2", target_bir_lowering=False, debug=False,
                   num_devices=NCORES)
    xT = nc.declare_dram_parameter("xT", [H // P, P, C], _DT, isOutput=False)
    W1 = nc.declare_dram_parameter("W1", [F // P, P, H // P, P], _DT, isOutput=False)
    W2 = nc.declare_dram_parameter(
        "W2", [H // P, F // (KC * P), P, KC, P], _DT, isOutput=False)
    b1 = nc.declare_dram_parameter("b1", [P, F // P], mybir.dt.float32,
                                   isOutput=False)
    b2 = nc.declare_dram_parameter("b2", [P, H // P], mybir.dt.float32,
                                   isOutput=False)
    if repeat > 1:
        salt = nc.declare_dram_parameter("salt", [1, repeat],
                                         mybir.dt.float32, isOutput=False)
    out = nc.declare_dram_parameter("out", [H, C], _DT, isOutput=True)

    KH = H // P    # 8
    KF = F // P    # 32
    MF = F // P    # 32
    MH = H // P    # 8
    NBLK = C // NB  # 2

    with tile.TileContext(nc) as tc, ExitStack() as ctx:
        xpool = ctx.enter_context(tc.tile_pool(name="xpool", bufs=1))
        h1pool = ctx.enter_context(tc.tile_pool(name="h1pool", bufs=1))
        cpool = ctx.enter_context(tc.tile_pool(name="cpool", bufs=1))
        wpool = ctx.enter_context(tc.tile_pool(name="wpool", bufs=16))
        ypool = ctx.enter_context(tc.tile_pool(name="ypool", bufs=4))
        psum = ctx.enter_context(tc.tile_pool(name="psum", bufs=7, space="PSUM"))
        wpsum = ctx.enter_context(tc.tile_pool(name="wpsum", bufs=1, space="PSUM"))

        if repeat > 1:
            salt_sb = cpool.tile([1, repeat], mybir.dt.float32, name="salt_sb")
            nc.sync.dma_start(out=salt_sb[:], in_=salt[:])

        # PE warm-up: tiny matmuls on a memset scratch keep the PE busy while
        # the first x/W1 DMAs are in flight, so the HAM clock-gate is at 8/8
        # when the real stream starts. One-time (outside the repeat loop).
        if warmup > 0:
            warm_sb = cpool.tile([P, 64], _DT, name="warm_sb")
            nc.vector.memset(warm_sb[:], 0.5)
            wps = wpsum.tile([64, 64], mybir.dt.float32, name="wps")
            for _ in range(warmup):
                nc.tensor.matmul(out=wps[:], lhsT=warm_sb[:, :64],
                                 rhs=warm_sb[:, :64], start=True, stop=True)

        loop_ctx = tc.For_i(0, repeat, 1) if repeat > 1 else None
        if loop_ctx is not None:
            loop_ctx.__enter__()

        # --- startup: x in two half-column waves across three queues, W1
        # slab 0 first on scalar so the PE can start ASAP ---
        x_sb = xpool.tile([P, KH, C], _DT, name="x_sb")
        w1_slabs = {}

        w1s0 = wpool.tile([P, KH, P], _DT, tag="wslab", name="w1s")
        nc.scalar.dma_start(out=w1s0[:], in_=W1[0])
        w1_slabs[0] = w1s0

        x_eng = [nc.sync, nc.scalar, nc.gpsimd]
        for k in range(KH):
            x_eng[k % 3].dma_start(out=x_sb[:, k, 0:NB], in_=xT[k, :, 0:NB])

        w1s1 = wpool.tile([P, KH, P], _DT, tag="wslab", name="w1s")
        nc.sync.dma_start(out=w1s1[:], in_=W1[1])
        w1_slabs[1] = w1s1

        for k in range(KH):
            x_eng[(k + 1) % 3].dma_start(out=x_sb[:, k, NB:C], in_=xT[k, :, NB:C])

        b1_sb = cpool.tile([P, MF], mybir.dt.float32, name="b1_sb")
        nc.gpsimd.dma_start(out=b1_sb[:], in_=b1[:])
        b2_sb = cpool.tile([P, MH], mybir.dt.float32, name="b2_sb")
        nc.gpsimd.dma_start(out=b2_sb[:], in_=b2[:])

        h1_sb = h1pool.tile([P, KF, C], _DT, name="h1_sb")

        # mm1: h1[m*P+p, c] = relu(b1[m*P+p] + sum_h W1[h, m*P+p] * xT[h, c])
        # The two n-blocks interleave in the k loop sharing one lhsT load.
        for m in range(MF):
            if m in w1_slabs:
                w1s = w1_slabs[m]
            else:
                w1s = wpool.tile([P, KH, P], _DT, tag="wslab", name="w1s")
                eng = nc.scalar if (m % 2 == 0) else nc.sync
                eng.dma_start(out=w1s[:], in_=W1[m])
            pss = [psum.tile([P, NB], mybir.dt.float32, tag="ps", name="ps")
                   for _ in range(NBLK)]
            for k in range(KH):
                for n in range(NBLK):
                    nc.tensor.matmul(
                        out=pss[n][:],
                        lhsT=w1s[:, k, :],
                        rhs=x_sb[:, k, n * NB:(n + 1) * NB],
                        start=(k == 0),
                        stop=(k == KH - 1),
                    )
            for n in range(NBLK):
                nc.scalar.activation(
                    out=h1_sb[:, m, n * NB:(n + 1) * NB],
                    in_=pss[n][:],
                    func=mybir.ActivationFunctionType.Relu,
                    bias=b1_sb[:, m:m + 1],
                )

        # mm2: y[mh*P+p, c] = b2[mh*P+p] + sum_f W2[f, mh*P+p] * h1[f, c]
        for mh in range(MH):
            w2chunks = []
            for kc in range(KF // KC):
                w2s = wpool.tile([P, KC, P], _DT, tag="wslab", name="w2s")
                eng = nc.scalar if (kc % 2 == 0) else nc.sync
                eng.dma_start(out=w2s[:], in_=W2[mh, kc])
                w2chunks.append(w2s)
            last = (mh == MH - 1)
            # normal mh: 2 blocks of 512; final mh: 512/256/128/128 so the
            # tail bias-add/store chain after the very last matmul is short
            # and the last two store receipts overlap on the two HWDGE rings
            blocks = [(0, NB), (NB, NB)] if not last else \
                     [(0, NB), (NB, NB // 2), (NB + NB // 2, NB // 4),
                      (NB + 3 * NB // 4, NB // 4)]
            if not last:
                # blocks interleave in the k loop, sharing one lhsT load
                pss = [psum.tile([P, cw], mybir.dt.float32, tag="ps",
                                 name="ps2") for _, cw in blocks]
                for k in range(KF):
                    for bi, (c0, cw) in enumerate(blocks):
                        nc.tensor.matmul(
                            out=pss[bi][:],
                            lhsT=w2chunks[k // KC][:, k % KC, :],
                            rhs=h1_sb[:, k, c0:c0 + cw],
                            start=(k == 0),
                            stop=(k == KF - 1),
                        )
                for bi, (c0, cw) in enumerate(blocks):
                    y_sb = ypool.tile([P, cw], _DT, tag="y", name="y_sb")
                    nc.scalar.activation(
                        out=y_sb[:], in_=pss[bi][:],
                        func=mybir.ActivationFunctionType.Identity,
                        bias=b2_sb[:, mh:mh + 1],
                    )
                    # HWDGE rings, not SWDGE: the Q7 descriptor path costs
                    # ~1us first-byte and measured ~3us/iter slower overall
                    oeng = nc.sync if (mh + bi) % 2 == 0 else nc.scalar
                    oeng.dma_start(
                        out=out[mh * P:(mh + 1) * P, c0:c0 + cw], in_=y_sb[:])
            else:
                # final mh: sequential blocks so earlier stores drain while
                # later (smaller) blocks still compute; the 128-col stores
                # ride both HWDGE rings so their HBM receipts overlap
                for bi, (c0, cw) in enumerate(blocks):
                    ps2 = psum.tile([P, cw], mybir.dt.float32, tag="ps",
                                    name="ps2")
                    for k in range(KF):
                        nc.tensor.matmul(
                            out=ps2[:],
                            lhsT=w2chunks[k // KC][:, k % KC, :],
                            rhs=h1_sb[:, k, c0:c0 + cw],
                            start=(k == 0),
                            stop=(k == KF - 1),
                        )
                    y_sb = ypool.tile([P, cw], _DT, tag="y", name="y_sb")
                    nc.scalar.activation(
                        out=y_sb[:], in_=ps2[:],
                        func=mybir.ActivationFunctionType.Identity,
                        bias=b2_sb[:, mh:mh + 1],
                    )
                    oeng = nc.sync if bi % 2 == 0 else nc.scalar
                    oeng.dma_start(
                        out=out[mh * P:(mh + 1) * P, c0:c0 + cw], in_=y_sb[:])
        if loop_ctx is not None:
            loop_ctx.__exit__(None, None, None)
    nc.compile()
    return nc


_NC_CACHE = None


def _get_nc():
    global _NC_CACHE
    if _NC_CACHE is None:
        _NC_CACHE = _build_expert_ffn()
    return _NC_CACHE


def _route(tokens: np.ndarray, Wg: np.ndarray):
    """Top-1 gating with capacity C on jax-CPU, mirroring the reference 1:1
    so discrete routing decisions are bit-identical."""
    import jax
    import jax.numpy as jnp

    cpu = jax.devices("cpu")[0]
    with jax.default_device(cpu):
        tok = jnp.asarray(tokens)
        logits = tok @ jnp.asarray(Wg)
        gates = jax.nn.softmax(logits, axis=-1)
        idx = jnp.argmax(gates, axis=1)
        mask1 = jax.nn.one_hot(idx, E, dtype=gates.dtype)
        locations1 = jnp.cumsum(mask1, axis=0) - 1.0
        mask1 = mask1 * (locations1 < C).astype(gates.dtype)
        gates1 = jnp.sum(gates * mask1, axis=1)

        mask_np = np.asarray(mask1)
        gate_val = np.asarray(gates1, dtype=np.float32)

    tok_ids = [np.nonzero(mask_np[:, e] > 0)[0] for e in range(E)]
    return tok_ids, gate_val


def _prep_in_maps(x, Wg, W1, b1, W2, b2):
    import ml_dtypes
    bf16 = ml_dtypes.bfloat16

    tokens = x.reshape(T, H)
    tok_ids, gate_val = _route(tokens, Wg)

    in_maps = []
    for e in range(E):
        ids = tok_ids[e]
        # pad unused capacity slots with copies of real tokens (outputs of
        # padded slots are discarded below); all-zero columns measurably
        # slow the PE clock-gate on this silicon
        if len(ids) > 0:
            pad_ids = np.resize(ids, C)
        else:
            pad_ids = np.zeros(C, dtype=np.int64)
        xT_e = tokens[pad_ids].astype(bf16).T.copy()
        W1p = np.ascontiguousarray(
            W1[e].astype(bf16).reshape(H // P, P, F // P, P).transpose(2, 1, 0, 3))
        W2p = np.ascontiguousarray(
            W2[e].astype(bf16).reshape(
                F // (KC * P), KC, P, H // P, P).transpose(3, 0, 2, 1, 4))
        in_maps.append({
            "xT": np.ascontiguousarray(xT_e.reshape(H // P, P, C)),
            "W1": W1p,
            "W2": W2p,
            "b1": np.ascontiguousarray(b1[e].reshape(F // P, P).T),
            "b2": np.ascontiguousarray(b2[e].reshape(H // P, P).T),
        })
    return in_maps, tok_ids, gate_val


def kernel(x, Wg, W1, b1, W2, b2):
    x = np.asarray(x, dtype=np.float32)
    Wg = np.asarray(Wg, dtype=np.float32)
    W1 = np.asarray(W1, dtype=np.float32)
    b1 = np.asarray(b1, dtype=np.float32)
    W2 = np.asarray(W2, dtype=np.float32)
    b2 = np.asarray(b2, dtype=np.float32)

    in_maps, tok_ids, gate_val = _prep_in_maps(x, Wg, W1, b1, W2, b2)

    nc = _get_nc()
    res = run_bass_kernel_spmd(nc, in_maps, list(range(NCORES)))

    out = np.zeros((T, H), dtype=np.float32)
    for e in range(E):
        ids = tok_ids[e]
        yT = res.results[e]["out"]
        out[ids] = yT[:, :len(ids)].astype(np.float32).T * gate_val[ids, None]
    return out.reshape(B, S, H)
